# revision 36
# baseline (speedup 1.0000x reference)
"""Cross-attention (softmax over queries) on 8 Trainium2 NeuronCores.

Reference (per batch b):
    q = y @ Wq.T + bq            [N, H]
    k = x @ Wk.T + bk            [M, H]
    v = x @ Wv.T + bv            [M, D]
    dots = (q @ k.T) * H**-0.5   [N, M]
    attn = softmax(dots, axis=0) (over queries n, per key column m)
    out  = attn @ v              [N, D]

Sharding: data-parallel over batch B=8, one batch per core (SPMD).

Device algorithm (per core, all matmuls fp16 with fp32 PSUM accumulation):
  A. gpsimd DMA casts y,x to fp16 in flight; PE-transpose 128x128 blocks
     (identity matmul) into yT[c,n], xT[c,m]; project qT[h,n], kT[h,m]
     (weights arrive pre-transposed/pre-scaled fp16 from host; q/k biases
     added by the ACT psum->sbuf copy, per-partition).
  C. per 128-row key chunk mc: V-projection chunk (PE filler work, bias via
     K=1 matmul), dotsT[m,n] into two [128,1024] PSUM halves, column max
     (DVE), fused exp+rowsum on ACT into attnT fp16, fold 1/colsum into v.
  D. out[n,d] = sum_m attnT[m,n] * v'[m,d]; dense 16-matmul PSUM chains.
"""

from contextlib import ExitStack

import numpy as np

import concourse.mybir as mybir
import concourse.tile as tile
from concourse import bacc
from concourse.bass_utils import run_bass_kernel_spmd
from concourse.masks import make_identity

F32 = mybir.dt.float32
F16 = mybir.dt.float16
Exp = mybir.ActivationFunctionType.Exp
AX = mybir.AxisListType.X

B, N, M, C, H, D = 8, 2048, 2048, 1024, 512, 1024
P = 128
NT, MT, CCH, HC = N // P, M // P, C // P, H // P  # 16, 16, 8, 4
SCALE = (C // 2) ** -0.5

_CACHE = {}


def _build_nc():
    nc = bacc.Bacc("TRN2", target_bir_lowering=False, debug=False)

    y_d = nc.dram_tensor("y", [N, C], F32, kind="ExternalInput").ap()
    x_d = nc.dram_tensor("x", [M, C], F32, kind="ExternalInput").ap()
    wqt_d = nc.dram_tensor("wqt", [C, H], F16, kind="ExternalInput").ap()
    wkt_d = nc.dram_tensor("wkt", [C, H], F16, kind="ExternalInput").ap()
    wvt_d = nc.dram_tensor("wvt", [C, D], F16, kind="ExternalInput").ap()
    bq_d = nc.dram_tensor("bq", [H], F32, kind="ExternalInput").ap()
    bk_d = nc.dram_tensor("bk", [H], F32, kind="ExternalInput").ap()
    bv_d = nc.dram_tensor("bv", [D], F16, kind="ExternalInput").ap()
    out_d = nc.dram_tensor("out", [N, D], F32, kind="ExternalOutput").ap()

    y_r = y_d.rearrange("(t p) c -> p t c", p=P)  # [128, 16, 1024]
    x_r = x_d.rearrange("(t p) c -> p t c", p=P)
    out_r = out_d.rearrange("(t p) d -> p t d", p=P)

    with tile.TileContext(nc) as tc:
        with (
            tc.tile_pool(name="persist", bufs=1) as pers,
            tc.tile_pool(name="stats", bufs=1) as stats,
            tc.tile_pool(name="xT_pool", bufs=1) as xTp,
        ):
            # ps_pp spans phases A+C; closed explicitly before phase D
            pp_stack = ExitStack()
            psPP = pp_stack.enter_context(
                tc.tile_pool(name="ps_pp", bufs=4, space="PSUM")
            )
            qT = pers.tile([P, HC, N], F16, tag="qT")  # [h%128, h//128, n] 2MB
            kT = pers.tile([P, HC, M], F16, tag="kT")  # 2MB
            v = pers.tile([P, MT, D], F16, tag="v")  # [m%128, m//128, d] 4MB
            ones = pers.tile([1, 512], F16, tag="ones")
            nc.vector.memset(ones[:], 1.0)
            ident = pers.tile([P, P], F16, tag="ident")
            make_identity(nc, ident[:])

            sums = stats.tile([P, MT], F32, tag="sums")
            rsum = stats.tile([P, MT], F32, tag="rsum")
            bq_sb = stats.tile([P, HC], F32, tag="bq")  # [h%128, h//128]
            bk_sb = stats.tile([P, HC], F32, tag="bk")
            bv_sb = stats.tile([1, D], F16, tag="bv")
            nc.sync.dma_start(bq_sb[:], bq_d.rearrange("(o p) -> p o", p=P))
            nc.sync.dma_start(bk_sb[:], bk_d.rearrange("(o p) -> p o", p=P))
            nc.sync.dma_start(bv_sb[:], bv_d[None, :])

            xT = xTp.tile([P, CCH, M], F16, tag="xT")  # alive through phase C

            # ---------- Phase A: transposes + q/k projections ----------
            with (
                tc.tile_pool(name="stage_ld", bufs=4) as sld,
                tc.tile_pool(name="yT_pool", bufs=1) as yTp,
                tc.tile_pool(name="w_pool", bufs=1) as wp,
                tc.tile_pool(name="ps_tr", bufs=4, space="PSUM") as psTR,
            ):
                wq_sb = wp.tile([P, CCH, H], F16, tag="wq")  # [c%128, c//128, h]
                wk_sb = wp.tile([P, CCH, H], F16, tag="wk")
                nc.sync.dma_start(wq_sb[:], wqt_d.rearrange("(o p) h -> p o h", p=P))
                nc.sync.dma_start(wk_sb[:], wkt_d.rearrange("(o p) h -> p o h", p=P))

                def load_transposed(src_r, dst, scope, chunks=(4, 4, 4, 4)):
                    # src_r: DRAM [128, 16, 1024] f32; dst: SBUF [128, 8, 2048] f16
                    # gpsimd DMA converts f32->f16 in flight; PE transposes
                    # 128x128 blocks; ACT copies PSUM->SBUF.
                    with nc.named_scope(scope):
                        nt0 = 0
                        for sz in chunks:
                            a16 = sld.tile([P, 4, C], F16, tag="a16")
                            nc.gpsimd.dma_start(
                                out=a16[:, :sz, :],
                                in_=src_r[:, nt0 : nt0 + sz, :],
                            )
                            for cc in range(CCH):
                                ptr = psTR.tile([P, 512], F16, tag="tr")
                                for t in range(sz):
                                    nc.tensor.transpose(
                                        ptr[:, t * P : (t + 1) * P],
                                        a16[:, t, cc * P : (cc + 1) * P],
                                        ident[:],
                                    )
                                nc.scalar.copy(
                                    dst[:, cc, nt0 * P : (nt0 + sz) * P],
                                    ptr[:, : sz * P],
                                )
                            nt0 += sz

                def project(dst, w_sb, b_sb, src_T, scope):
                    with nc.named_scope(scope):
                        for hc in range(HC):
                            for j in range(N // 512):
                                pp = psPP.tile([P, 512], F32, tag="pp")
                                for cc in range(CCH):
                                    nc.tensor.matmul(
                                        pp[:],
                                        w_sb[:, cc, hc * P : (hc + 1) * P],
                                        src_T[:, cc, j * 512 : (j + 1) * 512],
                                        start=(cc == 0),
                                        stop=(cc == CCH - 1),
                                    )
                                # ACT copy: psum -> f16, + per-partition bias
                                nc.scalar.add(
                                    dst[:, hc, j * 512 : (j + 1) * 512],
                                    pp[:],
                                    b_sb[:, hc : hc + 1],
                                )

                yT = yTp.tile([P, CCH, N], F16, tag="yT")
                load_transposed(y_r, yT, "A_y", chunks=(1, 1, 2, 4, 4, 4))
                project(qT, wq_sb, bq_sb, yT, "A_qT")
                load_transposed(x_r, xT, "A_x")
                project(kT, wk_sb, bk_sb, xT, "A_kT")

            # ---------- Phase C: V-proj chunks interleaved with dots/softmax ----------
            with (
                tc.tile_pool(name="late", bufs=1) as late,
                tc.tile_pool(name="sc", bufs=4) as sc,
            ):
                psC_stack = ExitStack()
                psC = psC_stack.enter_context(
                    tc.tile_pool(name="ps_c", bufs=1, space="PSUM")
                )
                attnT = late.tile([P, MT, N], F16, tag="attnT")  # 8MB
                wv_sb = late.tile([P, CCH, D], F16, tag="wv")  # 2MB
                nc.sync.dma_start(wv_sb[:], wvt_d.rearrange("(o p) d -> p o d", p=P))

                def v_chunk(mc):
                    # v[m, d] for m-chunk mc: lhsT = xT (c,m), rhs = wv (c,d)
                    for dh in range(2):
                        pv = psPP.tile([P, 512], F32, tag="pp")
                        for cc in range(CCH):
                            nc.tensor.matmul(
                                pv[:],
                                xT[:, cc, mc * P : (mc + 1) * P],
                                wv_sb[:, cc, dh * 512 : (dh + 1) * 512],
                                start=(cc == 0),
                                stop=False,
                            )
                        nc.tensor.matmul(
                            pv[:],
                            ones[:, :P],
                            bv_sb[:, dh * 512 : (dh + 1) * 512],
                            start=False,
                            stop=True,
                        )
                        nc.scalar.copy(v[:, mc, dh * 512 : (dh + 1) * 512], pv[:])

                def dots_chunk(mc):
                    halves = []
                    for h in range(2):
                        pd = psC.tile([P, 1024], F32, tag=f"dots{h}")
                        for j2 in range(2):
                            j = h * 2 + j2
                            for hc in range(HC):
                                nc.tensor.matmul(
                                    pd[:, j2 * 512 : (j2 + 1) * 512],
                                    kT[:, hc, mc * P : (mc + 1) * P],
                                    qT[:, hc, j * 512 : (j + 1) * 512],
                                    start=(hc == 0),
                                    stop=(hc == HC - 1),
                                )
                        halves.append(pd)
                    pmax = sc.tile([P, 4], F32, tag="pmax")
                    for h in range(2):
                        for j2 in range(2):
                            nc.vector.reduce_max(
                                pmax[:, 2 * h + j2 : 2 * h + j2 + 1],
                                halves[h][:, j2 * 512 : (j2 + 1) * 512],
                                axis=AX,
                            )
                    negmax = sc.tile([P, 1], F32, tag="negmax")
                    nc.vector.reduce_max(negmax[:], pmax[:], axis=AX, negate=True)
                    ssum = sc.tile([P, 2], F32, tag="ssum")
                    for h in range(2):
                        nc.scalar.activation(
                            out=attnT[:, mc, h * 1024 : (h + 1) * 1024],
                            in_=halves[h][:],
                            func=Exp,
                            bias=negmax[:],
                            accum_out=ssum[:, h : h + 1],
                        )
                    nc.vector.tensor_tensor(
                        sums[:, mc : mc + 1],
                        ssum[:, 0:1],
                        ssum[:, 1:2],
                        mybir.AluOpType.add,
                    )
                    nc.vector.reciprocal(rsum[:, mc : mc + 1], sums[:, mc : mc + 1])
                    # fold 1/colsum into v rows for this m-chunk
                    nc.vector.tensor_tensor(
                        v[:, mc, :],
                        v[:, mc, :],
                        rsum[:, mc : mc + 1].to_broadcast((P, D)),
                        mybir.AluOpType.mult,
                    )

                with nc.named_scope("C_loop"):
                    for mc in range(MT):
                        v_chunk(mc)
                        dots_chunk(mc)
                psC_stack.close()
                pp_stack.close()

                # ---------- Phase D: out = attnT^T @ v' ----------
                with (
                    tc.tile_pool(name="ps_d", bufs=4, space="PSUM") as psD,
                    tc.tile_pool(name="so", bufs=4) as so,
                    nc.named_scope("D_out"),
                ):
                    for ntc in range(NT):
                        for dh in range(2):
                            po = psD.tile([P, 512], F32, tag="po")
                            for mc in range(MT):
                                nc.tensor.matmul(
                                    po[:],
                                    attnT[:, mc, ntc * P : (ntc + 1) * P],
                                    v[:, mc, dh * 512 : (dh + 1) * 512],
                                    start=(mc == 0),
                                    stop=(mc == MT - 1),
                                )
                            ot = so.tile([P, 512], F32, tag="ot")
                            nc.scalar.copy(ot[:], po[:])
                            nc.sync.dma_start(
                                out_r[:, ntc, dh * 512 : (dh + 1) * 512], ot[:]
                            )

    nc.finalize()
    return nc


def _get_nc():
    if "nc" not in _CACHE:
        _CACHE["nc"] = _build_nc()
    return _CACHE["nc"]


def _prep_in_maps(y, x, Wq, bq, Wk, bk, Wv, bv):
    y = np.ascontiguousarray(np.asarray(y, dtype=np.float32))
    x = np.ascontiguousarray(np.asarray(x, dtype=np.float32))
    wqt = np.ascontiguousarray((np.asarray(Wq) * SCALE).T.astype(np.float16))
    wkt = np.ascontiguousarray(np.asarray(Wk).T.astype(np.float16))
    wvt = np.ascontiguousarray(np.asarray(Wv).T.astype(np.float16))
    bq32 = (np.asarray(bq) * SCALE).astype(np.float32)
    bk32 = np.asarray(bk, dtype=np.float32)
    bv16 = np.asarray(bv).astype(np.float16)
    return [
        {
            "y": y[b],
            "x": x[b],
            "wqt": wqt,
            "wkt": wkt,
            "wvt": wvt,
            "bq": bq32,
            "bk": bk32,
            "bv": bv16,
        }
        for b in range(B)
    ]


def run(inputs, trace=False, trace_cores=None):
    nc = _get_nc()
    in_maps = _prep_in_maps(**inputs)
    r = run_bass_kernel_spmd(
        nc, in_maps, list(range(B)), trace=trace, trace_cores=trace_cores
    )
    out = np.stack([r.results[b]["out"] for b in range(B)], axis=0)
    return out, r


def kernel(**inputs) -> np.ndarray:
    out, _ = run(inputs, trace=False)
    return out


# revision 39
# speedup vs baseline: 1.1453x; 1.1453x over previous
"""Cross-attention (softmax over queries) on 8 Trainium2 NeuronCores.

Reference (per batch b):
    q = y @ Wq.T + bq            [N, H]
    k = x @ Wk.T + bk            [M, H]
    v = x @ Wv.T + bv            [M, D]
    dots = (q @ k.T) * H**-0.5   [N, M]
    attn = softmax(dots, axis=0) (over queries n, per key column m)
    out  = attn @ v              [N, D]

Sharding: data-parallel over batch B=8, one batch per core (SPMD).

Device algorithm (per core, all matmuls fp16 with fp32 PSUM accumulation):
  A. gpsimd DMA casts y,x to fp16 in flight; PE-transpose 128x128 blocks
     (identity matmul) into yT[c,n], xT[c,m]; project qT[h,n], kT[h,m]
     (weights arrive pre-transposed/pre-scaled fp16 from host; q/k biases
     added by the ACT psum->sbuf copy, per-partition).
  C. per 128-row key chunk mc: V-projection chunk (PE filler work, bias via
     K=1 matmul), dotsT[m,n] into two [128,1024] PSUM halves, column max
     (DVE), fused exp+rowsum on ACT into attnT fp16, fold 1/colsum into v.
  D. out[n,d] = sum_m attnT[m,n] * v'[m,d]; dense 16-matmul PSUM chains.
"""

from contextlib import ExitStack

import numpy as np

import concourse.mybir as mybir
import concourse.tile as tile
from concourse import bacc
from concourse.bass_utils import run_bass_kernel_spmd
from concourse.masks import make_identity

F32 = mybir.dt.float32
F16 = mybir.dt.float16
Exp = mybir.ActivationFunctionType.Exp
AX = mybir.AxisListType.X

B, N, M, C, H, D = 8, 2048, 2048, 1024, 512, 1024
P = 128
NT, MT, CCH, HC = N // P, M // P, C // P, H // P  # 16, 16, 8, 4
SCALE = (C // 2) ** -0.5

_CACHE = {}


def _build_nc():
    nc = bacc.Bacc("TRN2", target_bir_lowering=False, debug=False)

    y_d = nc.dram_tensor("y", [N, C], F32, kind="ExternalInput").ap()
    x_d = nc.dram_tensor("x", [M, C], F32, kind="ExternalInput").ap()
    wqt_d = nc.dram_tensor("wqt", [C, H], F16, kind="ExternalInput").ap()
    wkt_d = nc.dram_tensor("wkt", [C, H], F16, kind="ExternalInput").ap()
    wvt_d = nc.dram_tensor("wvt", [C, D], F16, kind="ExternalInput").ap()
    bq_d = nc.dram_tensor("bq", [H], F32, kind="ExternalInput").ap()
    bk_d = nc.dram_tensor("bk", [H], F32, kind="ExternalInput").ap()
    bv_d = nc.dram_tensor("bv", [D], F16, kind="ExternalInput").ap()
    out_d = nc.dram_tensor("out", [N, D], F32, kind="ExternalOutput").ap()

    y_r = y_d.rearrange("(t p) c -> p t c", p=P)  # [128, 16, 1024]
    x_r = x_d.rearrange("(t p) c -> p t c", p=P)
    out_r = out_d.rearrange("(t p) d -> p t d", p=P)

    with tile.TileContext(nc) as tc:
        with (
            tc.tile_pool(name="persist", bufs=1) as pers,
            tc.tile_pool(name="stats", bufs=1) as stats,
            tc.tile_pool(name="xT_pool", bufs=1) as xTp,
        ):
            # ps_pp spans phases A+C; closed explicitly before phase D
            pp_stack = ExitStack()
            psPP = pp_stack.enter_context(
                tc.tile_pool(name="ps_pp", bufs=4, space="PSUM")
            )
            qT = pers.tile([P, HC, N], F16, tag="qT")  # [h%128, h//128, n] 2MB
            kT = pers.tile([P, HC, M], F16, tag="kT")  # 2MB
            v = pers.tile([P, MT, D], F16, tag="v")  # [m%128, m//128, d] 4MB
            ones = pers.tile([1, 512], F16, tag="ones")
            nc.vector.memset(ones[:], 1.0)
            ident = pers.tile([P, P], F16, tag="ident")
            make_identity(nc, ident[:])

            sums = stats.tile([P, MT], F32, tag="sums")
            rsum = stats.tile([P, MT], F32, tag="rsum")
            bq_sb = stats.tile([P, HC], F32, tag="bq")  # [h%128, h//128]
            bk_sb = stats.tile([P, HC], F32, tag="bk")
            bv_sb = stats.tile([1, D], F16, tag="bv")
            nc.sync.dma_start(bq_sb[:], bq_d.rearrange("(o p) -> p o", p=P))
            nc.sync.dma_start(bk_sb[:], bk_d.rearrange("(o p) -> p o", p=P))
            nc.sync.dma_start(bv_sb[:], bv_d[None, :])

            xT = xTp.tile([P, CCH, M], F16, tag="xT")  # alive through phase C

            # ---------- Phase A: transposes + q/k projections ----------
            with (
                tc.tile_pool(name="stage_ld", bufs=4) as sld,
                tc.tile_pool(name="yT_pool", bufs=1) as yTp,
                tc.tile_pool(name="w_pool", bufs=1) as wp,
                tc.tile_pool(name="ps_tr", bufs=4, space="PSUM") as psTR,
            ):
                wq_sb = wp.tile([P, CCH, H], F16, tag="wq")  # [c%128, c//128, h]
                wk_sb = wp.tile([P, CCH, H], F16, tag="wk")
                nc.sync.dma_start(wq_sb[:], wqt_d.rearrange("(o p) h -> p o h", p=P))
                nc.sync.dma_start(wk_sb[:], wkt_d.rearrange("(o p) h -> p o h", p=P))

                def transpose_chunk(src_r, dst, nt0, sz):
                    # load sz row-tiles of src (f32), cast to f16 in the DMA,
                    # PE-transpose into dst[:, :, nt0*128 : (nt0+sz)*128]
                    a16 = sld.tile([P, 4, C], F16, tag="a16")
                    nc.gpsimd.dma_start(
                        out=a16[:, :sz, :], in_=src_r[:, nt0 : nt0 + sz, :]
                    )
                    for cc in range(CCH):
                        ptr = psTR.tile([P, 512], F16, tag="tr")
                        for t in range(sz):
                            nc.tensor.transpose(
                                ptr[:, t * P : (t + 1) * P],
                                a16[:, t, cc * P : (cc + 1) * P],
                                ident[:],
                            )
                        nc.scalar.copy(
                            dst[:, cc, nt0 * P : (nt0 + sz) * P], ptr[:, : sz * P]
                        )

                def project(dst, w_sb, b_sb, src_T, scope):
                    with nc.named_scope(scope):
                        for hc in range(HC):
                            for j in range(N // 512):
                                pp = psPP.tile([P, 512], F32, tag="pp")
                                for cc in range(CCH):
                                    nc.tensor.matmul(
                                        pp[:],
                                        w_sb[:, cc, hc * P : (hc + 1) * P],
                                        src_T[:, cc, j * 512 : (j + 1) * 512],
                                        start=(cc == 0),
                                        stop=(cc == CCH - 1),
                                    )
                                # ACT copy: psum -> f16, + per-partition bias
                                nc.scalar.add(
                                    dst[:, hc, j * 512 : (j + 1) * 512],
                                    pp[:],
                                    b_sb[:, hc : hc + 1],
                                )

                yT = yTp.tile([P, CCH, N], F16, tag="yT")
                with nc.named_scope("A_y"):
                    for nt0, sz in [(0, 4), (4, 4), (8, 4), (12, 4)]:
                        transpose_chunk(y_r, yT, nt0, sz)
                project(qT, wq_sb, bq_sb, yT, "A_qT")
                with nc.named_scope("A_x"):
                    for nt0, sz in [(0, 4), (4, 4), (8, 4), (12, 4)]:
                        transpose_chunk(x_r, xT, nt0, sz)
                project(kT, wk_sb, bk_sb, xT, "A_kT")

            # ---------- Phase C: V-proj chunks interleaved with dots/softmax ----------
            with (
                tc.tile_pool(name="late", bufs=1) as late,
                tc.tile_pool(name="sc", bufs=4) as sc,
            ):
                psC_stack = ExitStack()
                psC = psC_stack.enter_context(
                    tc.tile_pool(name="ps_c", bufs=1, space="PSUM")
                )
                attnT = late.tile([P, MT, N], F16, tag="attnT")  # 8MB
                wv_sb = late.tile([P, CCH, D], F16, tag="wv")  # 2MB
                nc.sync.dma_start(wv_sb[:], wvt_d.rearrange("(o p) d -> p o d", p=P))

                def v_chunk(mc):
                    # v[m, d] for m-chunk mc: lhsT = xT (c,m), rhs = wv (c,d)
                    for dh in range(2):
                        pv = psPP.tile([P, 512], F32, tag="pp")
                        for cc in range(CCH):
                            nc.tensor.matmul(
                                pv[:],
                                xT[:, cc, mc * P : (mc + 1) * P],
                                wv_sb[:, cc, dh * 512 : (dh + 1) * 512],
                                start=(cc == 0),
                                stop=False,
                            )
                        nc.tensor.matmul(
                            pv[:],
                            ones[:, :P],
                            bv_sb[:, dh * 512 : (dh + 1) * 512],
                            start=False,
                            stop=True,
                        )
                        nc.scalar.copy(v[:, mc, dh * 512 : (dh + 1) * 512], pv[:])

                def dots_chunk(mc):
                    halves = []
                    for h in range(2):
                        pd = psC.tile([P, 1024], F32, tag=f"dots{h}")
                        for j2 in range(2):
                            j = h * 2 + j2
                            for hc in range(HC):
                                nc.tensor.matmul(
                                    pd[:, j2 * 512 : (j2 + 1) * 512],
                                    kT[:, hc, mc * P : (mc + 1) * P],
                                    qT[:, hc, j * 512 : (j + 1) * 512],
                                    start=(hc == 0),
                                    stop=(hc == HC - 1),
                                )
                        halves.append(pd)
                    pmax = sc.tile([P, 4], F32, tag="pmax")
                    for h in range(2):
                        for j2 in range(2):
                            nc.vector.reduce_max(
                                pmax[:, 2 * h + j2 : 2 * h + j2 + 1],
                                halves[h][:, j2 * 512 : (j2 + 1) * 512],
                                axis=AX,
                            )
                    negmax = sc.tile([P, 1], F32, tag="negmax")
                    nc.vector.reduce_max(negmax[:], pmax[:], axis=AX, negate=True)
                    ssum = sc.tile([P, 2], F32, tag="ssum")
                    for h in range(2):
                        nc.scalar.activation(
                            out=attnT[:, mc, h * 1024 : (h + 1) * 1024],
                            in_=halves[h][:],
                            func=Exp,
                            bias=negmax[:],
                            accum_out=ssum[:, h : h + 1],
                        )
                    nc.vector.tensor_tensor(
                        sums[:, mc : mc + 1],
                        ssum[:, 0:1],
                        ssum[:, 1:2],
                        mybir.AluOpType.add,
                    )
                    nc.vector.reciprocal(rsum[:, mc : mc + 1], sums[:, mc : mc + 1])
                    # fold 1/colsum into v rows for this m-chunk
                    nc.vector.tensor_tensor(
                        v[:, mc, :],
                        v[:, mc, :],
                        rsum[:, mc : mc + 1].to_broadcast((P, D)),
                        mybir.AluOpType.mult,
                    )

                with nc.named_scope("C_loop"):
                    for mc in range(MT):
                        v_chunk(mc)
                        dots_chunk(mc)
                psC_stack.close()
                pp_stack.close()

                # ---------- Phase D: out = attnT^T @ v' ----------
                with (
                    tc.tile_pool(name="ps_d", bufs=4, space="PSUM") as psD,
                    tc.tile_pool(name="so", bufs=4) as so,
                    nc.named_scope("D_out"),
                ):
                    for ntc in range(NT):
                        for dh in range(2):
                            po = psD.tile([P, 512], F32, tag="po")
                            for mc in range(MT):
                                nc.tensor.matmul(
                                    po[:],
                                    attnT[:, mc, ntc * P : (ntc + 1) * P],
                                    v[:, mc, dh * 512 : (dh + 1) * 512],
                                    start=(mc == 0),
                                    stop=(mc == MT - 1),
                                )
                            ot = so.tile([P, 512], F32, tag="ot")
                            nc.scalar.copy(ot[:], po[:])
                            nc.sync.dma_start(
                                out_r[:, ntc, dh * 512 : (dh + 1) * 512], ot[:]
                            )

    nc.finalize()
    return nc


def _get_nc():
    if "nc" not in _CACHE:
        _CACHE["nc"] = _build_nc()
    return _CACHE["nc"]


def _prep_in_maps(y, x, Wq, bq, Wk, bk, Wv, bv):
    y = np.ascontiguousarray(np.asarray(y, dtype=np.float32))
    x = np.ascontiguousarray(np.asarray(x, dtype=np.float32))
    wqt = np.ascontiguousarray((np.asarray(Wq) * SCALE).T.astype(np.float16))
    wkt = np.ascontiguousarray(np.asarray(Wk).T.astype(np.float16))
    wvt = np.ascontiguousarray(np.asarray(Wv).T.astype(np.float16))
    bq32 = (np.asarray(bq) * SCALE).astype(np.float32)
    bk32 = np.asarray(bk, dtype=np.float32)
    bv16 = np.asarray(bv).astype(np.float16)
    return [
        {
            "y": y[b],
            "x": x[b],
            "wqt": wqt,
            "wkt": wkt,
            "wvt": wvt,
            "bq": bq32,
            "bk": bk32,
            "bv": bv16,
        }
        for b in range(B)
    ]


def run(inputs, trace=False, trace_cores=None):
    nc = _get_nc()
    in_maps = _prep_in_maps(**inputs)
    r = run_bass_kernel_spmd(
        nc, in_maps, list(range(B)), trace=trace, trace_cores=trace_cores
    )
    out = np.stack([r.results[b]["out"] for b in range(B)], axis=0)
    return out, r


def kernel(**inputs) -> np.ndarray:
    out, _ = run(inputs, trace=False)
    return out
